# revision 2
# baseline (speedup 1.0000x reference)
"""ConvFace GNN message-passing kernel for 8 Trainium2 NeuronCores.

Reference computation (per mesh m):
  x[:, j]  = fea[:, pool_idx[j]] + sum_k fea[:, ring_n[m, j, k]]   # gather + K-sum
  y        = W @ x + b                                             # 1x1 conv
  y_norm   = BN(y) (training-mode batch stats over all meshes+faces), ReLU

Sharding: data-parallel over meshes — core m handles mesh m (M == 8 == n_cores).
BN batch statistics are globally all-reduced across the 8 cores on-device.

Device algorithm per core (all fp32 exact):
  - 17 indirect-DMA gathers (128 rows each) per 128-face group: gt[p, s, :] =
    feaT[src(face=g*128+p, s), :]   (faces land on partitions)
  - DVE strided reduce over s -> x_rows [128 faces, C]
  - TensorE transpose -> x^T [C, 128] in PSUM, evicted to x_sb [C, FP]
  - TensorE fp32 matmuls: y = (W^T)^T @ x  (O=256 in two 128-row halves)
  - ScalarE evicts y PSUM->SBUF with accum_out partial sums; Square pass for
    sum(y^2); per-channel partials all-reduced across cores (2KB collective)
  - note: the conv bias b cancels in training-mode BN (shift-invariant), so it
    is never applied
  - BN scale/shift folded into one ScalarE relu(s*y + t) pass, DMA out
"""
import sys

sys.path.insert(0, "/opt/trn_rl_repo")

import numpy as np

from concourse import bass, bacc, tile, mybir
from concourse.bass_utils import run_bass_kernel_spmd

# Problem shape (hardcoded per contest contract)
M = 8          # meshes == cores
C = 128        # input channels
O = 256        # output channels
F = 16384      # faces
FP = 8192      # pooled faces
K = 16         # neighbors
K17 = K + 1    # neighbors + self
BN_EPS = 1e-5
N_CORES = 8
NGROUPS = FP // 128          # 64 groups of 128 output faces
NQ = 4                       # SWDGE queues for gather round-robin
FP32 = mybir.dt.float32


def _build_program(reps: int = 1):
    """Build the Bass program. `reps` repeats the whole per-mesh pipeline
    (for timing amplification in test harnesses; kernel output uses rep 0...
    reps-1 all write the same results)."""
    nc = bacc.Bacc("TRN2", target_bir_lowering=False, debug=False,
                   num_devices=N_CORES, num_swdge_queues=NQ)

    feaT_in = nc.dram_tensor("feaT", [F, C], FP32, kind="ExternalInput")
    idx_in = nc.dram_tensor("idx", [128, NGROUPS * K17], mybir.dt.int32,
                            kind="ExternalInput")
    Wt_in = nc.dram_tensor("Wt", [128, O], FP32, kind="ExternalInput")
    gb_in = nc.dram_tensor("gb", [128, 4], FP32, kind="ExternalInput")
    ident_in = nc.dram_tensor("ident", [128, 128], FP32, kind="ExternalInput")
    y_out = nc.dram_tensor("y", [O, FP], FP32, kind="ExternalOutput")

    cc_in = nc.dram_tensor("cc_in", [128, 4], FP32)
    cc_out = nc.dram_tensor("cc_out", [128, 4], FP32, addr_space="Shared")

    NTOT = float(M * FP)  # BN normalizer (biased stats over meshes+faces)

    with tile.TileContext(nc) as tc:
        with tc.tile_pool(name="cpool", bufs=1) as cpool, \
             tc.tile_pool(name="gbuf", bufs=6) as gpool, \
             tc.tile_pool(name="xr", bufs=3) as xrpool, \
             tc.tile_pool(name="ot", bufs=3) as opool, \
             tc.tile_pool(name="scr", bufs=2) as scrpool, \
             tc.tile_pool(name="xps", bufs=2, space="PSUM") as xpsum, \
             tc.tile_pool(name="yps", bufs=2, space="PSUM") as ypsum:
            idx_sb = cpool.tile([128, NGROUPS * K17], mybir.dt.int32)
            nc.sync.dma_start(out=idx_sb[:], in_=idx_in[:])
            Wt_sb = cpool.tile([128, O], FP32)
            nc.sync.dma_start(out=Wt_sb[:], in_=Wt_in[:])
            gb_sb = cpool.tile([128, 4], FP32)
            nc.sync.dma_start(out=gb_sb[:], in_=gb_in[:])
            id_sb = cpool.tile([128, 128], FP32)
            nc.sync.dma_start(out=id_sb[:], in_=ident_in[:])

            x_sb = cpool.tile([128, FP], FP32)
            y_sb = cpool.tile([128, 2, FP], FP32)
            acc_y = cpool.tile([128, 2, 16], FP32)
            acc_y2 = cpool.tile([128, 2, 16], FP32)
            sums = cpool.tile([128, 4], FP32)
            gsum = cpool.tile([128, 4], FP32)
            st_sb = cpool.tile([128, 8], FP32)  # mean/ex2/var/s/t slots [2 each]

            for _rep in range(reps):
                # ---- gather + K-sum + transpose: x_sb[C, FP] ----
                for g in range(NGROUPS):
                    gt = gpool.tile([128, K17, C], FP32, tag="g")
                    for s in range(K17):
                        q = g * K17 + s
                        bi = nc.gpsimd.indirect_dma_start(
                            out=gt[:, s, :],
                            out_offset=None,
                            in_=feaT_in[:],
                            in_offset=bass.IndirectOffsetOnAxis(
                                ap=idx_sb[:, q:q + 1], axis=0),
                        )
                        qn = q % NQ
                        if qn:
                            bi.ins.queue = f"qPoolDynamic{qn}"
                    xr = xrpool.tile([128, C], FP32, tag="xr")
                    nc.vector.tensor_reduce(
                        out=xr[:],
                        in_=gt[:].rearrange("p s c -> p c s"),
                        axis=mybir.AxisListType.X,
                        op=mybir.AluOpType.add,
                    )
                    xps = xpsum.tile([128, 128], FP32, tag="xps")
                    nc.tensor.transpose(out=xps[:], in_=xr[:], identity=id_sb[:])
                    nc.scalar.copy(
                        out=x_sb[:, g * 128:(g + 1) * 128], in_=xps[:])

                # ---- y = Wt.T @ x, evict + stats ----
                for h in range(2):
                    for jb in range(16):
                        yps = ypsum.tile([128, 512], FP32, tag="yps")
                        nc.tensor.matmul(
                            yps[:],
                            Wt_sb[:, h * 128:(h + 1) * 128],
                            x_sb[:, jb * 512:(jb + 1) * 512],
                            start=True,
                            stop=True,
                        )
                        nc.scalar.activation(
                            out=y_sb[:, h, jb * 512:(jb + 1) * 512],
                            in_=yps[:],
                            func=mybir.ActivationFunctionType.Copy,
                            accum_out=acc_y[:, h, jb:jb + 1],
                        )
                        scr = scrpool.tile([128, 512], FP32, tag="scr")
                        nc.scalar.activation(
                            out=scr[:],
                            in_=yps[:],
                            func=mybir.ActivationFunctionType.Square,
                            accum_out=acc_y2[:, h, jb:jb + 1],
                        )

                # ---- local stat partials -> collective all-reduce ----
                nc.vector.tensor_reduce(
                    out=sums[:, 0:2], in_=acc_y[:],
                    axis=mybir.AxisListType.X, op=mybir.AluOpType.add)
                nc.vector.tensor_reduce(
                    out=sums[:, 2:4], in_=acc_y2[:],
                    axis=mybir.AxisListType.X, op=mybir.AluOpType.add)
                nc.sync.dma_start(out=cc_in[:], in_=sums[:])
                nc.gpsimd.collective_compute(
                    "AllReduce",
                    mybir.AluOpType.add,
                    replica_groups=[list(range(N_CORES))],
                    ins=[cc_in[:]],
                    outs=[cc_out[:]],
                )
                nc.sync.dma_start(out=gsum[:], in_=cc_out[:])

                # ---- scale/shift: s = gamma/sqrt(var+eps), t = beta - mean*s
                # st_sb slots: [0:2]=mean  [2:4]=ex2  [4:6]=s  [6:8]=t
                nc.scalar.mul(st_sb[:, 0:2], gsum[:, 0:2], 1.0 / NTOT)
                nc.scalar.mul(st_sb[:, 2:4], gsum[:, 2:4], 1.0 / NTOT)
                # var = ex2 - mean^2  (into st_sb[:,2:4]);  sd = sqrt(var+eps)
                nc.vector.tensor_tensor(
                    out=st_sb[:, 4:6], in0=st_sb[:, 0:2], in1=st_sb[:, 0:2],
                    op=mybir.AluOpType.mult)
                nc.vector.tensor_tensor(
                    out=st_sb[:, 2:4], in0=st_sb[:, 2:4], in1=st_sb[:, 4:6],
                    op=mybir.AluOpType.subtract)
                nc.vector.tensor_scalar_add(st_sb[:, 2:4], st_sb[:, 2:4],
                                            BN_EPS)
                nc.scalar.activation(
                    out=st_sb[:, 2:4], in_=st_sb[:, 2:4],
                    func=mybir.ActivationFunctionType.Sqrt)
                nc.vector.reciprocal(out=st_sb[:, 4:6], in_=st_sb[:, 2:4])
                # s = gamma * inv
                nc.vector.tensor_tensor(
                    out=st_sb[:, 4:6], in0=st_sb[:, 4:6], in1=gb_sb[:, 0:2],
                    op=mybir.AluOpType.mult)
                # t = beta - mean * s
                nc.vector.tensor_tensor(
                    out=st_sb[:, 0:2], in0=st_sb[:, 0:2], in1=st_sb[:, 4:6],
                    op=mybir.AluOpType.mult)
                nc.vector.tensor_tensor(
                    out=st_sb[:, 6:8], in0=gb_sb[:, 2:4], in1=st_sb[:, 0:2],
                    op=mybir.AluOpType.subtract)

                # ---- apply BN + ReLU, store ----
                for h in range(2):
                    for cb in range(8):
                        ot = opool.tile([128, 1024], FP32, tag="ot")
                        nc.scalar.activation(
                            out=ot[:],
                            in_=y_sb[:, h, cb * 1024:(cb + 1) * 1024],
                            func=mybir.ActivationFunctionType.Relu,
                            scale=st_sb[:, 4 + h:5 + h],
                            bias=st_sb[:, 6 + h:7 + h],
                        )
                        nc.sync.dma_start(
                            out=y_out[h * 128:(h + 1) * 128,
                                      cb * 1024:(cb + 1) * 1024],
                            in_=ot[:])

    nc.compile()
    return nc


def _prep_inputs(fea, W, b, gamma, beta, ring_n, pool_idx):
    """Host-side marshalling into per-core input maps."""
    fea = np.asarray(fea, dtype=np.float32)
    W = np.asarray(W, dtype=np.float32)
    gamma = np.asarray(gamma, dtype=np.float32)
    beta = np.asarray(beta, dtype=np.float32)
    ring_n = np.asarray(ring_n)
    pool_idx = np.asarray(pool_idx)

    Wt = np.ascontiguousarray(W.T)                       # [C=128, O]
    gb = np.stack([gamma[:128], gamma[128:],
                   beta[:128], beta[128:]], axis=1).astype(np.float32)
    ident = np.eye(128, dtype=np.float32)

    in_maps = []
    for m in range(M):
        feaT = np.ascontiguousarray(fea[m].T)            # [F, C]
        # idx[p, g*17+s]: source face for output face j=g*128+p, source s
        arr = np.concatenate(
            [pool_idx[:, None], ring_n[m]], axis=1)       # [FP, 17]
        idx = np.ascontiguousarray(
            arr.reshape(NGROUPS, 128, K17).transpose(1, 0, 2)
            .reshape(128, NGROUPS * K17)).astype(np.int32)
        in_maps.append({
            "feaT": feaT, "idx": idx, "Wt": Wt, "gb": gb, "ident": ident,
        })
    return in_maps


_CACHED_NC = None


def kernel(fea, W, b, gamma, beta, ring_n, pool_idx):
    """Full-input entry point: returns BN(ReLU(conv(gather-sum))) [M, O, FP]."""
    global _CACHED_NC
    if _CACHED_NC is None:
        _CACHED_NC = _build_program(reps=1)
    nc = _CACHED_NC
    in_maps = _prep_inputs(fea, W, b, gamma, beta, ring_n, pool_idx)
    res = run_bass_kernel_spmd(nc, in_maps, list(range(N_CORES)))
    out = np.stack([res.results[m]["y"] for m in range(M)], axis=0)
    return out.astype(np.float32)
